# revision 5
# baseline (speedup 1.0000x reference)
"""Attention-gate block (conv1x1+BN x2 -> relu -> conv1x1+BN -> sigmoid -> mul)
on 8 TRN2 NeuronCores, data-parallel over batch with sync-BN via tiny AllReduces.

Self-contained: hardcodes shapes B=16, C=256, F=128, H=W=64, 8 cores.
"""
import numpy as np

import concourse.bacc as bacc
import concourse.mybir as mybir
import concourse.tile as tile
from concourse.bass_utils import run_bass_kernel_spmd

F32 = mybir.dt.float32
F32R = mybir.dt.float32r
AF = mybir.ActivationFunctionType
OP = mybir.AluOpType

N_CORES = 8
B, C, F, HW = 16, 256, 128, 64 * 64        # full batch, channels, F_int, pixels/sample
SPC = B // N_CORES                          # samples per core = 2
NPIX = SPC * HW                             # pixels per core = 8192
NTOT = B * HW                               # global BN count = 65536
EPS = 1e-5
NT = HW // 512                              # 512-pixel tiles per sample = 8

MM_DT = F32                                 # main conv matmul dtype
PSI_DT = F32                                # psi conv matmul dtype


def _build():
    nc = bacc.Bacc(trn_type="TRN2", target_bir_lowering=False, debug=False,
                   num_devices=N_CORES)
    g_d = nc.dram_tensor("g_sh", [SPC, C, HW], F32, kind="ExternalInput")
    x_d = nc.dram_tensor("x_sh", [SPC, C, HW], F32, kind="ExternalInput")
    wgT_d = nc.dram_tensor("wgT", [C, F], F32, kind="ExternalInput")
    wxT_d = nc.dram_tensor("wxT", [C, F], F32, kind="ExternalInput")
    psiw_d = nc.dram_tensor("psiw", [F, 1], F32, kind="ExternalInput")
    gb_d = nc.dram_tensor("gb", [F, 4], F32, kind="ExternalInput")
    psigb_d = nc.dram_tensor("psigb", [1, 4], F32, kind="ExternalInput")
    out_d = nc.dram_tensor("out_sh", [SPC, C, HW], F32, kind="ExternalOutput")

    with tile.TileContext(nc) as tc:
        _body(nc, tc, g_d, x_d, wgT_d, wxT_d, psiw_d, gb_d, psigb_d, out_d)
    nc.finalize()
    return nc


def _body(nc, tc, g_d, x_d, wgT_d, wxT_d, psiw_d, gb_d, psigb_d, out_d):
    from contextlib import ExitStack
    es = ExitStack()
    const = es.enter_context(tc.tile_pool(name="const", bufs=1))
    xpool = es.enter_context(tc.tile_pool(name="xdata", bufs=1))
    gpool = es.enter_context(tc.tile_pool(name="gdata", bufs=2))
    ypool = es.enter_context(tc.tile_pool(name="ydata", bufs=1))
    spool = es.enter_context(tc.tile_pool(name="stats", bufs=1))
    zpool = es.enter_context(tc.tile_pool(name="zscratch", bufs=3))
    upool = es.enter_context(tc.tile_pool(name="udata", bufs=1))
    dram = es.enter_context(tc.tile_pool(name="drambounce", bufs=1, space="DRAM"))

    # ---- constants / weights into SBUF ----
    w_g = const.tile([128, 2, 128], F32, tag="w_g")
    nc.sync.dma_start(w_g[:], wgT_d[:].rearrange("(k c) o -> c k o", k=2))
    w_x = const.tile([128, 2, 128], F32, tag="w_x")
    nc.sync.dma_start(w_x[:], wxT_d[:].rearrange("(k c) o -> c k o", k=2))
    psiw = const.tile([128, 1], F32, tag="psiw")
    nc.sync.dma_start(psiw[:], psiw_d[:])
    gb = const.tile([128, 4], F32, tag="gb")
    nc.sync.dma_start(gb[:], gb_d[:])
    psigb = const.tile([1, 4], F32, tag="psigb")
    nc.sync.dma_start(psigb[:], psigb_d[:])
    ones_all = const.tile([128, 128], F32, tag="ones_all")
    nc.vector.memset(ones_all[:], 1.0)
    ones_col = const.tile([128, 1], F32, tag="ones_col")
    nc.vector.memset(ones_col[:], 1.0)
    epsc = const.tile([128, 1], F32, tag="epsc")
    nc.vector.memset(epsc[:], EPS)

    # ---- persistent big buffers ----
    x_t = [[xpool.tile([128, HW], F32, tag=f"x_{s}_{k}", name=f"x_{s}_{k}")
            for k in range(2)] for s in range(SPC)]
    y_g = ypool.tile([128, NPIX], F32, tag="y_g")   # conv-g out; later relu(z); later out stage s0
    y_x = ypool.tile([128, NPIX], F32, tag="y_x")   # conv-x out; later out stage s1

    acc_bn_g = spool.tile([128, SPC * NT * 6], F32, tag="abn_g")
    acc_bn_x = spool.tile([128, SPC * NT * 6], F32, tag="abn_x")
    bn2 = spool.tile([128, 4], F32, tag="bn2")      # aggregated (mean,var) g|x
    S = spool.tile([128, 4], F32, tag="S")          # local (S1,S2) g|x
    R = spool.tile([128, 4], F32, tag="R")          # allreduced
    pv = spool.tile([128, 16], F32, tag="pv")       # param scratch columns
    prm = spool.tile([128, 4], F32, tag="prm")      # s_g, s_x, tsum
    u_ar = spool.tile([1, 8], F32, tag="u_ar")      # psi stats payload
    r2 = spool.tile([1, 8], F32, tag="r2")
    ab = spool.tile([1, 2], F32, tag="ab")          # A, B scalars
    ab128 = spool.tile([128, 2], F32, tag="ab128")

    # psi-conv out lives on partitions {0,32,64,96}, row = 32*(s*2+half)
    u_sb = upool.tile([128, 2048], F32, tag="u_sb")
    psi_sb = upool.tile([128, 2048], F32, tag="psi_sb")  # u^2 scratch, then sigmoid

    in_b = dram.tile([128, 4], F32, tag="ar1_in")
    out_b = dram.tile([128, 4], F32, tag="ar1_out")
    in2_b = dram.tile([1, 8], F32, tag="ar2_in")
    out2_b = dram.tile([1, 8], F32, tag="ar2_out")

    # ================= Phase A: load + conv matmuls + local stats =================
    with tc.tile_pool(name="psumA", bufs=8, space="PSUM") as psA:
        g_t = {}
        for s in range(SPC):
            for k in range(2):
                gt = gpool.tile([128, HW], F32, tag="gld")
                nc.sync.dma_start(gt[:], g_d[s, k * 128:(k + 1) * 128, :])
                g_t[(s, k)] = gt
            for k in range(2):
                nc.sync.dma_start(x_t[s][k][:], x_d[s, k * 128:(k + 1) * 128, :])
            for tname, wt, src, ysb, abn in (
                ("g", w_g, [g_t[(s, 0)], g_t[(s, 1)]], y_g, acc_bn_g),
                ("x", w_x, [x_t[s][0], x_t[s][1]], y_x, acc_bn_x),
            ):
                for j in range(NT):
                    js = slice(j * 512, (j + 1) * 512)
                    ps = psA.tile([128, 512], F32, tag="psA")
                    nc.tensor.matmul(ps[:], w_ap(wt, 0, MM_DT),
                                     src[0][:, js].bitcast(MM_DT),
                                     start=True, stop=False)
                    nc.tensor.matmul(ps[:], w_ap(wt, 1, MM_DT),
                                     src[1][:, js].bitcast(MM_DT),
                                     start=False, stop=True)
                    nc.scalar.activation(ysb[:, s * HW + j * 512: s * HW + (j + 1) * 512],
                                         ps[:], AF.Copy)
                    t = s * NT + j
                    nc.vector.bn_stats(abn[:, t * 6:(t + 1) * 6], ps[:])

    # local (mean,var) -> (S1,S2) in S[:, 0:2]=g, [:, 2:4]=x
    nc.vector.bn_aggr(bn2[:, 0:2], acc_bn_g[:])
    nc.vector.bn_aggr(bn2[:, 2:4], acc_bn_x[:])
    for i in range(2):  # 0=g, 1=x
        m = bn2[:, 2 * i:2 * i + 1]
        v = bn2[:, 2 * i + 1:2 * i + 2]
        nc.vector.tensor_scalar(S[:, 2 * i:2 * i + 1], m, float(NPIX), None, OP.mult)
        # S2 = (m*m + v) * NPIX
        nc.vector.scalar_tensor_tensor(pv[:, 15:16], m, m, v, OP.mult, OP.add)
        nc.vector.tensor_scalar(S[:, 2 * i + 1:2 * i + 2], pv[:, 15:16], float(NPIX),
                                None, OP.mult)

    # ================= AllReduce 1: [128,4] channel sums =================
    nc.sync.dma_start(in_b[:], S[:])
    nc.gpsimd.collective_compute("AllReduce", OP.add,
                                 replica_groups=[list(range(N_CORES))],
                                 ins=[in_b.opt()], outs=[out_b.opt()])
    nc.sync.dma_start(R[:], out_b[:])

    # ================= Phase B: per-channel affine params =================
    # cols of pv: 0 mean,1 msq,2 negvar,3 std,4 inv,5 m*s, per path
    invn = 1.0 / float(NTOT)
    for i, (gcol, bcol, scol) in enumerate(((0, 1, 0), (2, 3, 1))):  # (gamma,beta in gb, out col in prm)
        base = 5 * i
        mean = pv[:, base + 0:base + 1]
        msq = pv[:, base + 1:base + 2]
        negv = pv[:, base + 2:base + 3]
        std = pv[:, base + 3:base + 4]
        inv = pv[:, base + 4:base + 5]
        nc.vector.tensor_scalar(mean, R[:, 2 * i:2 * i + 1], invn, None, OP.mult)
        nc.vector.tensor_scalar(msq, R[:, 2 * i + 1:2 * i + 2], invn, None, OP.mult)
        nc.vector.scalar_tensor_tensor(negv, mean, mean, msq, OP.mult, OP.subtract)
        nc.scalar.activation(std, negv, AF.Sqrt, bias=epsc[:], scale=-1.0)
        nc.vector.reciprocal(inv, std)
        nc.vector.tensor_mul(prm[:, scol:scol + 1], inv, gb[:, 2 * i:2 * i + 1])
        # t = beta - mean*s  (stash t_g in pv[:,10], t_x in pv[:,11])
        nc.vector.tensor_mul(pv[:, 12:13], mean, prm[:, scol:scol + 1])
        nc.vector.tensor_sub(pv[:, 10 + i:11 + i], gb[:, 2 * i + 1:2 * i + 2], pv[:, 12:13])
    nc.vector.tensor_add(prm[:, 2:3], pv[:, 10:11], pv[:, 11:12])  # tsum

    # ================= Phase C: z = s_g*y_g + s_x*y_x + tsum; r=relu; psi conv =================
    nc.vector.memset(u_sb[:], 0.0)
    with tc.tile_pool(name="psumU", bufs=2, space="PSUM") as psU:
        for s in range(SPC):
            for half in range(2):
                pu = psU.tile([1, 2048], F32, tag="psU")
                for q in range(4):
                    j = half * 4 + q
                    js = slice(s * HW + j * 512, s * HW + (j + 1) * 512)
                    h = zpool.tile([128, 512], F32, tag="h")
                    nc.vector.tensor_scalar(h[:], y_x[:, js], prm[:, 1:2], prm[:, 2:3],
                                            OP.mult, OP.add)
                    z = zpool.tile([128, 512], F32, tag="z")
                    nc.vector.scalar_tensor_tensor(z[:], y_g[:, js], prm[:, 0:1], h[:],
                                                   OP.mult, OP.add)
                    nc.scalar.activation(y_g[:, js], z[:], AF.Relu)
                    nc.tensor.matmul(pu[0:1, q * 512:(q + 1) * 512],
                                     psiw[:].bitcast(PSI_DT),
                                     y_g[:, js].bitcast(PSI_DT),
                                     start=True, stop=True)
                row = 32 * (s * 2 + half)
                nc.scalar.activation(u_sb[row:row + 1, :], pu[:], AF.Copy)

    # u stats: colsum over partitions via ones-matmul (zero rows contribute 0)
    nc.vector.tensor_mul(psi_sb[:], u_sb[:], u_sb[:])  # u^2 (psi_sb as scratch)
    with tc.tile_pool(name="psumS", bufs=2, space="PSUM") as psS:
        pus = psS.tile([1, 512], F32, tag="pus")
        pus2 = psS.tile([1, 512], F32, tag="pus2")
        for b in range(4):
            bs = slice(b * 512, (b + 1) * 512)
            nc.tensor.matmul(pus[:], ones_col[:], u_sb[:, bs], start=(b == 0), stop=(b == 3))
        for b in range(4):
            bs = slice(b * 512, (b + 1) * 512)
            nc.tensor.matmul(pus2[:], ones_col[:], psi_sb[:, bs], start=(b == 0), stop=(b == 3))
        nc.vector.memset(u_ar[:], 0.0)
        nc.vector.tensor_reduce(u_ar[0:1, 0:1], pus[:], mybir.AxisListType.X, OP.add)
        nc.vector.tensor_reduce(u_ar[0:1, 1:2], pus2[:], mybir.AxisListType.X, OP.add)

    # ================= AllReduce 2: psi stats =================
    nc.sync.dma_start(in2_b[:], u_ar[:])
    nc.gpsimd.collective_compute("AllReduce", OP.add,
                                 replica_groups=[list(range(N_CORES))],
                                 ins=[in2_b.opt()], outs=[out2_b.opt()])
    nc.sync.dma_start(r2[:], out2_b[:])

    # psi affine scalars A, B on partition 0
    pm = spool.tile([1, 8], F32, tag="pm")
    nc.vector.tensor_scalar(pm[0:1, 0:1], r2[0:1, 0:1], invn, None, OP.mult)  # mean
    nc.vector.tensor_scalar(pm[0:1, 1:2], r2[0:1, 1:2], invn, None, OP.mult)  # msq
    nc.vector.scalar_tensor_tensor(pm[0:1, 2:3], pm[0:1, 0:1], pm[0:1, 0:1],
                                   pm[0:1, 1:2], OP.mult, OP.subtract)        # -var
    nc.scalar.activation(pm[0:1, 3:4], pm[0:1, 2:3], AF.Sqrt, bias=epsc[0:1, :], scale=-1.0)
    nc.vector.reciprocal(pm[0:1, 4:5], pm[0:1, 3:4])
    nc.vector.tensor_mul(ab[0:1, 0:1], pm[0:1, 4:5], psigb[0:1, 0:1])         # A
    nc.vector.tensor_mul(pm[0:1, 5:6], pm[0:1, 0:1], ab[0:1, 0:1])
    nc.vector.tensor_sub(ab[0:1, 1:2], psigb[0:1, 1:2], pm[0:1, 5:6])         # B

    # ================= Phase E: sigmoid, broadcast, multiply, store =================
    nc.gpsimd.partition_broadcast(ab128[:], ab[:])
    nc.scalar.activation(psi_sb[:], u_sb[:], AF.Sigmoid,
                         bias=ab128[:, 1:2], scale=ab128[:, 0:1])

    stage = [y_g, y_x]
    with tc.tile_pool(name="psumB", bufs=4, space="PSUM") as psB:
        for s in range(SPC):
            for j in range(NT):
                row = 32 * (s * 2 + j // 4)
                cs = slice((j % 4) * 512, (j % 4 + 1) * 512)
                pb = psB.tile([128, 512], F32, tag="pb")
                nc.tensor.matmul(pb[:], ones_all[row:row + 1, :], psi_sb[row:row + 1, cs],
                                 start=True, stop=True, tile_position=(row, 0))
                for k in range(2):
                    ds = slice(k * HW + j * 512, k * HW + (j + 1) * 512)
                    xs = slice(j * 512, (j + 1) * 512)
                    nc.vector.tensor_mul(stage[s][:, ds], x_t[s][k][:, xs], pb[:])
            for k in range(2):
                nc.sync.dma_start(out_d[s, k * 128:(k + 1) * 128, :],
                                  stage[s][:, k * HW:(k + 1) * HW])
    es.close()


def w_ap(wtile, k, dt):
    return wtile[:, k, :].bitcast(dt)


_NC_CACHE = []


def kernel(**inputs):
    g = np.ascontiguousarray(np.asarray(inputs["g"], np.float32)).reshape(B, C, HW)
    x = np.ascontiguousarray(np.asarray(inputs["x"], np.float32)).reshape(B, C, HW)
    wgT = np.ascontiguousarray(np.asarray(inputs["wg_w"], np.float32).T)
    wxT = np.ascontiguousarray(np.asarray(inputs["wx_w"], np.float32).T)
    psiw = np.ascontiguousarray(np.asarray(inputs["psi_w"], np.float32).reshape(1, F).T)
    gb = np.ascontiguousarray(np.stack([
        np.asarray(inputs["wg_gamma"], np.float32),
        np.asarray(inputs["wg_beta"], np.float32),
        np.asarray(inputs["wx_gamma"], np.float32),
        np.asarray(inputs["wx_beta"], np.float32)], axis=1))
    psigb = np.array([[float(np.asarray(inputs["psi_gamma"]).reshape(-1)[0]),
                       float(np.asarray(inputs["psi_beta"]).reshape(-1)[0]), 0.0, 0.0]],
                     np.float32)

    if not _NC_CACHE:
        _NC_CACHE.append(_build())
    nc = _NC_CACHE[0]

    in_maps = []
    for i in range(N_CORES):
        sl = slice(i * SPC, (i + 1) * SPC)
        in_maps.append({"g_sh": g[sl], "x_sh": x[sl], "wgT": wgT, "wxT": wxT,
                        "psiw": psiw, "gb": gb, "psigb": psigb})
    res = run_bass_kernel_spmd(nc, in_maps, list(range(N_CORES)))
    out = np.concatenate([res.results[i]["out_sh"] for i in range(N_CORES)], axis=0)
    return out.reshape(B, C, 64, 64)


def run_traced(**inputs):
    """Like kernel() but with NTFF tracing; returns (out, BassKernelResults)."""
    g = np.ascontiguousarray(np.asarray(inputs["g"], np.float32)).reshape(B, C, HW)
    x = np.ascontiguousarray(np.asarray(inputs["x"], np.float32)).reshape(B, C, HW)
    wgT = np.ascontiguousarray(np.asarray(inputs["wg_w"], np.float32).T)
    wxT = np.ascontiguousarray(np.asarray(inputs["wx_w"], np.float32).T)
    psiw = np.ascontiguousarray(np.asarray(inputs["psi_w"], np.float32).reshape(1, F).T)
    gb = np.ascontiguousarray(np.stack([
        np.asarray(inputs["wg_gamma"], np.float32),
        np.asarray(inputs["wg_beta"], np.float32),
        np.asarray(inputs["wx_gamma"], np.float32),
        np.asarray(inputs["wx_beta"], np.float32)], axis=1))
    psigb = np.array([[float(np.asarray(inputs["psi_gamma"]).reshape(-1)[0]),
                       float(np.asarray(inputs["psi_beta"]).reshape(-1)[0]), 0.0, 0.0]],
                     np.float32)
    if not _NC_CACHE:
        _NC_CACHE.append(_build())
    nc = _NC_CACHE[0]
    in_maps = []
    for i in range(N_CORES):
        sl = slice(i * SPC, (i + 1) * SPC)
        in_maps.append({"g_sh": g[sl], "x_sh": x[sl], "wgT": wgT, "wxT": wxT,
                        "psiw": psiw, "gb": gb, "psigb": psigb})
    res = run_bass_kernel_spmd(nc, in_maps, list(range(N_CORES)), trace=True)
    out = np.concatenate([res.results[i]["out_sh"] for i in range(N_CORES)], axis=0)
    return out.reshape(B, C, 64, 64), res
